# revision 16
# baseline (speedup 1.0000x reference)
"""Trainium2 Bass kernel for the capsule-routing layer (nn_CapsConvLayer).

Math (reference):
  u_j_i[b,i,c,o] = sum_k W[i,c,o,k] * x[b,k,i]
  b_ic = 0
  3x:  c = softmax(b, axis=i)
       s[b,c,o]  = sum_i c[i,c] * u_j_i[b,i,c,o]
       out       = squash_over_c(s)
       agr[i,c]  = sum_{b,o} u_j_i[b,i,c,o] * out[b,c,o] / B
       b += agr
  return out[..., None]

Strategy: shard the input-capsule dim I across 8 cores (I_loc=144).  u_j_i is
never materialized; s is computed as one fused matmul with contraction over
(i,k) per core:  s~[b,co] = sum_ik u1[ik,b] * (exp(b)[i,c] * W1[ik,co]).
The softmax denominator Z and the partial s~ are all-reduced together
(the only cross-core traffic; 2 all-reduces total since the 3rd iteration's
agreement update is dead).  agr is computed locally per core via
T[co,ik] = sum_b out[b,co]*u2[b,ik] (PE), M = W2 .* T (DVE), then two small
PE contractions (class-selector and k-sum-replicate) put agr[i,c] directly
into the k-replicated [(i,k), c] layout used by the next iteration's weights.
The final iteration only needs partial s~3 + Z3, which the host sums and
squashes (tiny [256,10,16]).
"""

import os
import sys

sys.path.insert(0, "/opt/trn_rl_repo")

import ml_dtypes
import numpy as np

BF = ml_dtypes.bfloat16

import concourse.bacc as bacc
import concourse.mybir as mybir
import concourse.tile as tile
from concourse import bass_utils

# Problem constants (hardcoded per contract)
B, K, I, C, O = 256, 8, 1152, 10, 16
NCORES = 8
ILOC = I // NCORES          # 144
IK = ILOC * K               # 1152 contraction size per core
NT = IK // 128              # 9 partition tiles
CO = C * O                  # 160
NB = B // 128               # 2 batch chunks
F32 = mybir.dt.float32
BF16 = mybir.dt.bfloat16
ADD = mybir.AluOpType.add
MULT = mybir.AluOpType.mult

_CACHE = {}


def _build():
    nc = bacc.Bacc("TRN2", target_bir_lowering=False, debug=False,
                   enable_asserts=False, num_devices=NCORES)

    u1d = nc.dram_tensor("u1", [IK, B], BF16, kind="ExternalInput")
    u2d = nc.dram_tensor("u2", [B, IK], BF16, kind="ExternalInput")
    w1d = nc.dram_tensor("w1", [IK, CO], BF16, kind="ExternalInput")
    w2d = nc.dram_tensor("w2", [CO, IK], BF16, kind="ExternalInput")
    rmatd = nc.dram_tensor("rmat", [128, 128], BF16, kind="ExternalInput")
    seld = nc.dram_tensor("sel", [CO, C], BF16, kind="ExternalInput")
    s3d = nc.dram_tensor("s3out", [B, CO], F32, kind="ExternalOutput")
    z3d = nc.dram_tensor("z3out", [C, 1], F32, kind="ExternalOutput")

    with tile.TileContext(nc) as tc:
        with (
            tc.tile_pool(name="sb", bufs=1) as sb,
            tc.tile_pool(name="ps", bufs=2, space="PSUM") as ps,
            tc.tile_pool(name="ps2", bufs=2, space="PSUM") as ps2,
            tc.tile_pool(name="dram", bufs=1, space="DRAM") as dram,
        ):
            # persistent SBUF state
            u1sb = sb.tile([128, NT * B], BF16, tag="u1sb")     # [p, t*256+b]
            u2sb = sb.tile([128, NB * IK], BF16, tag="u2sb")    # [p, nb*1152+ik]
            w1sb = sb.tile([128, NT * CO], BF16, tag="w1sb")    # [p, t*160+co]
            w2sb = [sb.tile([80, IK], BF16, name=f"w2sb{g}", tag=f"w2sb{g}") for g in range(2)]
            cwsb = sb.tile([128, NT * CO], BF16, tag="cwsb")
            bsb = sb.tile([128, NT * C], F32, tag="bsb")       # logits, k-replicated
            cexp = sb.tile([128, NT * C], BF16, tag="cexp")
            onesb = sb.tile([128, 1], BF16, tag="onesb")
            rsb = sb.tile([128, 128], BF16, tag="rsb")
            selsb = [sb.tile([80, C], BF16, name=f"selsb{g}", tag=f"selsb{g}") for g in range(2)]
            ssb = sb.tile([128, NB * CO], BF16, tag="ssb")      # s~ global post-AR
            zbc = sb.tile([128, C], BF16, tag="zbc")
            qsb = sb.tile([128, C], F32, tag="qsb")
            outc = sb.tile([128, NB * CO], BF16, tag="outc")    # squashed caps
            agrk_sb = sb.tile([128, NT * C], BF16, tag="agrk_sb")
            msb = [sb.tile([80, IK], BF16, name=f"msb{g}", tag=f"msb{g}") for g in range(2)]
            zrow = sb.tile([1, CO], BF16, tag="zrow")
            spre = sb.tile([128, NB * CO], BF16, tag="spre")    # pre-AR s~ staging
            zpre = sb.tile([C, 1], BF16, tag="zpre")
            sq = sb.tile([128, NB * CO], BF16, tag="sq")
            n2 = sb.tile([128, NB * O], F32, tag="n2")
            phi = sb.tile([128, NB * O], F32, tag="phi")
            actscr = sb.tile([1, 1], F32, tag="actscr")
            s3pre = sb.tile([128, NB * CO], F32, tag="s3pre")
            z3pre = sb.tile([C, 1], F32, tag="z3pre")

            arin = [dram.tile([B + 1, CO], BF16, name=f"arin{i}", tag=f"arin{i}") for i in range(2)]
            arout = [dram.tile([B + 1, CO], BF16, name=f"arout{i}", tag=f"arout{i}") for i in range(2)]

            # ---- loads: merged big DMAs spread across engine queues ----
            nc.sync.dma_start(
                u1sb[:].rearrange("p (t b) -> p t b", b=B),
                u1d[:, :].rearrange("(t p) b -> p t b", p=128))
            nc.gpsimd.dma_start(
                w1sb[:].rearrange("p (t f) -> p t f", f=CO),
                w1d[:, :].rearrange("(t p) f -> p t f", p=128))
            nc.scalar.dma_start(
                u2sb[:].rearrange("p (n f) -> p n f", f=IK),
                u2d[:, :].rearrange("(n p) f -> p n f", p=128))
            for g in range(2):
                nc.sync.dma_start(w2sb[g][:], w2d[g * 80:(g + 1) * 80, :])
                nc.gpsimd.dma_start(selsb[g][:], seld[g * 80:(g + 1) * 80, :])
            nc.scalar.dma_start(rsb[:], rmatd[:, :])
            nc.vector.memset(bsb[:], 0.0)
            nc.vector.memset(cexp[:], 1.0)
            nc.gpsimd.memset(onesb[:], 1.0)
            nc.gpsimd.memset(zrow[:], 0.0)
            nc.gpsimd.memset(qsb[:], 1.0 / float(I))   # iter-1 softmax 1/Z

            EXP = mybir.ActivationFunctionType.Exp
            SQUARE = mybir.ActivationFunctionType.Square
            SQRT = mybir.ActivationFunctionType.Sqrt

            def s_matmul(wt):
                stiles = [ps.tile([128, CO], F32, name="spsum", tag="pbig") for _ in range(NB)]
                for nb in range(NB):
                    for t in range(NT):
                        nc.tensor.matmul(
                            stiles[nb][:],
                            u1sb[:, t * B + nb * 128: t * B + (nb + 1) * 128],
                            wt[:, t * CO:(t + 1) * CO],
                            start=(t == 0), stop=(t == NT - 1))
                return stiles

            def z_matmul():
                z = ps2.tile([C, 1], F32, name="zpsum", tag="psmall")
                for t in range(NT):
                    nc.tensor.matmul(z[:], cexp[:, t * C:(t + 1) * C], onesb[:],
                                     start=(t == 0), stop=(t == NT - 1))
                return z

            def pack_and_allreduce(it, stiles, z):
                ain, aout = arin[it], arout[it]
                for nb in range(NB):
                    scol = slice(nb * CO, (nb + 1) * CO)
                    nc.vector.tensor_copy(spre[:, scol], stiles[nb][:])
                if z is not None:
                    nc.vector.tensor_copy(zpre[:], z[:])
                    nc.sync.dma_start(ain[B:B + 1, 0:C], zpre[:])
                    nc.sync.dma_start(ain[B:B + 1, C:CO], zrow[:, C:CO])
                else:
                    nc.sync.dma_start(ain[B:B + 1, :], zrow[:])
                nc.sync.dma_start(
                    ain[0:B, :].rearrange("(n p) f -> p n f", p=128),
                    spre[:].rearrange("p (n f) -> p n f", f=CO))
                # prefetch the Sqrt activation table while the collective
                # runs; actscr = spre[0,0]^2 ties it to pack time
                nc.vector.tensor_tensor(actscr[:], spre[0:1, 0:1],
                                        spre[0:1, 0:1], op=MULT)
                nc.scalar.activation(actscr[:], actscr[:], SQRT)
                nc.gpsimd.collective_compute(
                    "AllReduce", ADD,
                    replica_groups=[list(range(NCORES))],
                    ins=[ain[:].opt()], outs=[aout[:].opt()])
                # unpack (zbc first via gpsimd: q starts while ssb streams)
                if z is not None:
                    nc.gpsimd.dma_start(zbc[:],
                                        aout[B:B + 1, 0:C].broadcast_to([128, C]))
                nc.sync.dma_start(
                    ssb[:].rearrange("p (n f) -> p n f", f=CO),
                    aout[0:B, :].rearrange("(n p) f -> p n f", p=128))

            def squash(const_q=False):
                # q = NCORES / Z8_glob  (= 1/Z); wide single ops over both
                # batch chunks at once.  Iter 1: Z == I exactly, q preset.
                if not const_q:
                    nc.vector.reciprocal(qsb[:], zbc[:])
                    nc.vector.tensor_scalar_mul(qsb[:], qsb[:], float(NCORES))
                s4 = ssb[:].rearrange("p (n c o) -> p n c o", c=C, o=O)
                q4 = qsb[:].unsqueeze(1).unsqueeze(3).broadcast_to([128, NB, C, O])
                nc.vector.tensor_tensor(s4, s4, q4, op=MULT)      # s = s~/Z
                nc.vector.tensor_tensor(sq[:], ssb[:], ssb[:], op=MULT)
                nc.vector.tensor_reduce(
                    n2[:],
                    sq[:].rearrange("p (n c o) -> p n o c", c=C, o=O),
                    axis=mybir.AxisListType.X, op=ADD)
                # phi = n2 / ((1+n2)*(sqrt(n2)+1e-10))
                nc.scalar.activation(phi[:], n2[:], SQRT)
                nc.vector.tensor_scalar_add(phi[:], phi[:], 1e-10)
                nc.vector.scalar_tensor_tensor(
                    phi[:], n2[:], 1.0, phi[:], op0=ADD, op1=MULT)
                nc.vector.reciprocal(phi[:], phi[:])
                nc.vector.tensor_tensor(phi[:], phi[:], n2[:], op=MULT)
                p4 = phi[:].rearrange("p (n o) -> p n o", o=O) \
                           .unsqueeze(2).broadcast_to([128, NB, C, O])
                o4 = outc[:].rearrange("p (n c o) -> p n c o", c=C, o=O)
                nc.vector.tensor_tensor(o4, s4, p4, op=MULT)
                # prefetch the Exp table (used by agr tail) while PE runs T
                nc.scalar.activation(actscr[:], phi[0:1, 0:1], EXP)

            def agr_phase():
                # T[co,ik] = sum_b out[b,co] u2[b,ik]; M = W2 .* T
                for g in range(2):
                    Tg = ps.tile([80, IK], F32, name="Tpsum", tag="pbig")
                    for c0, cn in ((0, 512), (512, 512), (1024, 128)):
                        for nb in range(NB):
                            nc.tensor.matmul(
                                Tg[:, c0:c0 + cn],
                                outc[:, nb * CO + g * 80: nb * CO + (g + 1) * 80],
                                u2sb[:, nb * IK + c0: nb * IK + c0 + cn],
                                start=(nb == 0), stop=(nb == NB - 1))
                    nc.vector.tensor_tensor(msb[g][:], w2sb[g][:], Tg[:], op=MULT)
                # all 9 tiles' agr_k into ONE psum tile (disjoint col ranges),
                # then single wide ops for copy / k-sum / b update / exp / cW
                agrk = ps2.tile([128, NT * C], F32, name="agrkp", tag="psmall")
                for t in range(NT):
                    tcol = slice(t * C, (t + 1) * C)
                    for g in range(2):
                        nc.tensor.matmul(agrk[:, tcol],
                                         msb[g][:, t * 128:(t + 1) * 128],
                                         selsb[g][:], start=(g == 0), stop=(g == 1))
                nc.vector.tensor_copy(agrk_sb[:], agrk[:])
                repl = ps2.tile([128, NT * C], F32, name="replp", tag="psmall")
                nc.tensor.matmul(repl[:], rsb[:], agrk_sb[:])
                nc.vector.tensor_tensor(bsb[:], bsb[:], repl[:], op=ADD)
                nc.scalar.activation(cexp[:], bsb[:], EXP)
                cw4 = cwsb[:].rearrange("p (t c o) -> p t c o", c=C, o=O)
                w14 = w1sb[:].rearrange("p (t c o) -> p t c o", c=C, o=O)
                ce4 = cexp[:].rearrange("p (t c) -> p t c", c=C) \
                             .unsqueeze(3).broadcast_to([128, NT, C, O])
                nc.vector.tensor_tensor(cw4, w14, ce4, op=MULT)

            # ================= iteration 1 =================
            with nc.named_scope("it1_s"):
                stiles = s_matmul(w1sb)       # cexp == 1 -> cW == W1
            with nc.named_scope("it1_ar"):
                pack_and_allreduce(0, stiles, None)
            with nc.named_scope("it1_squash"):
                squash(const_q=True)
            with nc.named_scope("it1_agr"):
                agr_phase()
                z = z_matmul()
            # ================= iteration 2 =================
            with nc.named_scope("it2_s"):
                stiles = s_matmul(cwsb)
            with nc.named_scope("it2_ar"):
                pack_and_allreduce(1, stiles, z)
            with nc.named_scope("it2_squash"):
                squash()
            with nc.named_scope("it2_agr"):
                agr_phase()
                z = z_matmul()
            # ================= iteration 3 (s~ partial only) =================
            stiles = s_matmul(cwsb)
            for nb in range(NB):
                scol = slice(nb * CO, (nb + 1) * CO)
                nc.vector.tensor_copy(s3pre[:, scol], stiles[nb][:])
                nc.sync.dma_start(s3d[nb * 128:(nb + 1) * 128, :], s3pre[:, scol])
            nc.vector.tensor_copy(z3pre[:], z[:])
            nc.sync.dma_start(z3d[:, :], z3pre[:])

    nc.compile()
    return nc


def _get_nc():
    if "nc" not in _CACHE:
        _CACHE["nc"] = _build()
    return _CACHE["nc"]


def _host_inputs(x, weights):
    x = np.ascontiguousarray(x, dtype=np.float32)
    weights = np.ascontiguousarray(weights, dtype=np.float32)
    rmat = np.kron(np.eye(16, dtype=np.float32), np.ones((8, 8), np.float32))
    sel = np.zeros((CO, C), np.float32)
    for c in range(C):
        sel[c * O:(c + 1) * O, c] = 1.0 / B
    in_maps = []
    for m in range(NCORES):
        sl = slice(m * ILOC, (m + 1) * ILOC)
        xs = x[:, :, sl]                          # [B, K, ILOC]
        ws = weights[sl]                          # [ILOC, C, O, K]
        in_maps.append({
            "u1": np.ascontiguousarray(xs.transpose(2, 1, 0).reshape(IK, B)).astype(BF),
            "u2": np.ascontiguousarray(xs.transpose(0, 2, 1).reshape(B, IK)).astype(BF),
            "w1": np.ascontiguousarray(ws.transpose(0, 3, 1, 2).reshape(IK, CO)).astype(BF),
            "w2": np.ascontiguousarray(ws.transpose(1, 2, 0, 3).reshape(CO, IK)).astype(BF),
            "rmat": rmat.astype(BF),
            "sel": sel.astype(BF),
        })
    return in_maps


def kernel(x, weights):
    nc = _get_nc()
    in_maps = _host_inputs(x, weights)
    trace = bool(int(os.environ.get("KERNEL_TRACE", "0")))
    res = bass_utils.run_bass_kernel_spmd(
        nc, in_maps, core_ids=list(range(NCORES)), trace=trace)
    if trace and res.exec_time_ns is not None:
        print(f"HW exec time: {res.exec_time_ns} ns")
        _CACHE["exec_time_ns"] = res.exec_time_ns
        _CACHE["results"] = res
    s3 = np.zeros((B, CO), np.float64)
    z8 = np.zeros((C,), np.float64)
    for r in res.results:
        s3 += r["s3out"]
        z8 += r["z3out"][:, 0]
    s = (s3.reshape(B, C, O) / (z8 / NCORES)[None, :, None]).astype(np.float32)
    nsq = (s * s).sum(1, keepdims=True)
    out = s * (nsq / (1.0 + nsq)) / (np.sqrt(nsq) + 1e-10)
    return out[..., None].astype(np.float32)


# revision 18
# speedup vs baseline: 1.0305x; 1.0305x over previous
"""Trainium2 Bass kernel for the capsule-routing layer (nn_CapsConvLayer).

Math (reference):
  u_j_i[b,i,c,o] = sum_k W[i,c,o,k] * x[b,k,i]
  b_ic = 0
  3x:  c = softmax(b, axis=i)
       s[b,c,o]  = sum_i c[i,c] * u_j_i[b,i,c,o]
       out       = squash_over_c(s)
       agr[i,c]  = sum_{b,o} u_j_i[b,i,c,o] * out[b,c,o] / B
       b += agr
  return out[..., None]

Strategy: shard the input-capsule dim I across 8 cores (I_loc=144).  u_j_i is
never materialized; s is computed as one fused matmul with contraction over
(i,k) per core:  s~[b,co] = sum_ik u1[ik,b] * (exp(b)[i,c] * W1[ik,co]).
The softmax denominator Z and the partial s~ are all-reduced together
(the only cross-core traffic; 2 all-reduces total since the 3rd iteration's
agreement update is dead).  agr is computed locally per core via
T[co,ik] = sum_b out[b,co]*u2[b,ik] (PE), M = W2 .* T (DVE), then two small
PE contractions (class-selector and k-sum-replicate) put agr[i,c] directly
into the k-replicated [(i,k), c] layout used by the next iteration's weights.
The final iteration only needs partial s~3 + Z3, which the host sums and
squashes (tiny [256,10,16]).
"""

import os
import sys

sys.path.insert(0, "/opt/trn_rl_repo")

import ml_dtypes
import numpy as np

BF = ml_dtypes.bfloat16

import concourse.bacc as bacc
import concourse.mybir as mybir
import concourse.tile as tile
from concourse import bass_utils

# Problem constants (hardcoded per contract)
B, K, I, C, O = 256, 8, 1152, 10, 16
NCORES = 8
ILOC = I // NCORES          # 144
IK = ILOC * K               # 1152 contraction size per core
NT = IK // 128              # 9 partition tiles
CO = C * O                  # 160
NB = B // 128               # 2 batch chunks
F32 = mybir.dt.float32
BF16 = mybir.dt.bfloat16
ADD = mybir.AluOpType.add
MULT = mybir.AluOpType.mult

NDUMMY = 40

_CACHE = {}


def _build():
    nc = bacc.Bacc("TRN2", target_bir_lowering=False, debug=False,
                   enable_asserts=False, num_devices=NCORES)

    u1d = nc.dram_tensor("u1", [IK, B], BF16, kind="ExternalInput")
    u2d = nc.dram_tensor("u2", [B, IK], BF16, kind="ExternalInput")
    w1d = nc.dram_tensor("w1", [IK, CO], BF16, kind="ExternalInput")
    w2d = nc.dram_tensor("w2", [CO, IK], BF16, kind="ExternalInput")
    rmatd = nc.dram_tensor("rmat", [128, 128], BF16, kind="ExternalInput")
    seld = nc.dram_tensor("sel", [CO, C], BF16, kind="ExternalInput")
    s3d = nc.dram_tensor("s3out", [B, CO], F32, kind="ExternalOutput")
    z3d = nc.dram_tensor("z3out", [C, 1], F32, kind="ExternalOutput")

    with tile.TileContext(nc) as tc:
        with (
            tc.tile_pool(name="sb", bufs=1) as sb,
            tc.tile_pool(name="ps", bufs=2, space="PSUM") as ps,
            tc.tile_pool(name="ps2", bufs=2, space="PSUM") as ps2,
            tc.tile_pool(name="dram", bufs=1, space="DRAM") as dram,
        ):
            # persistent SBUF state
            u1sb = sb.tile([128, NT * B], BF16, tag="u1sb")     # [p, t*256+b]
            u2sb = sb.tile([128, NB * IK], BF16, tag="u2sb")    # [p, nb*1152+ik]
            w1sb = sb.tile([128, NT * CO], BF16, tag="w1sb")    # [p, t*160+co]
            w2sb = [sb.tile([80, IK], BF16, name=f"w2sb{g}", tag=f"w2sb{g}") for g in range(2)]
            cwsb = sb.tile([128, NT * CO], BF16, tag="cwsb")
            bsb = sb.tile([128, NT * C], F32, tag="bsb")       # logits, k-replicated
            cexp = sb.tile([128, NT * C], BF16, tag="cexp")
            onesb = sb.tile([128, 1], BF16, tag="onesb")
            rsb = sb.tile([128, 128], BF16, tag="rsb")
            selsb = [sb.tile([80, C], BF16, name=f"selsb{g}", tag=f"selsb{g}") for g in range(2)]
            ssb = sb.tile([128, NB * CO], BF16, tag="ssb")      # s~ global post-AR
            zbc = sb.tile([128, C], BF16, tag="zbc")
            qsb = sb.tile([128, C], F32, tag="qsb")
            outc = sb.tile([128, NB * CO], BF16, tag="outc")    # squashed caps
            agrk_sb = sb.tile([128, NT * C], BF16, tag="agrk_sb")
            msb = [sb.tile([80, IK], BF16, name=f"msb{g}", tag=f"msb{g}") for g in range(2)]
            zrow = sb.tile([1, CO], BF16, tag="zrow")
            spre = sb.tile([128, NB * CO], BF16, tag="spre")    # pre-AR s~ staging
            zpre = sb.tile([C, 1], BF16, tag="zpre")
            sq = sb.tile([128, NB * CO], BF16, tag="sq")
            n2 = sb.tile([128, NB * O], F32, tag="n2")
            phi = sb.tile([128, NB * O], F32, tag="phi")
            actscr = sb.tile([1, 1], F32, tag="actscr")
            s3pre = sb.tile([128, NB * CO], F32, tag="s3pre")
            z3pre = sb.tile([C, 1], F32, tag="z3pre")

            arin = [dram.tile([B + 1, CO], BF16, name=f"arin{i}", tag=f"arin{i}") for i in range(2)]
            arout = [dram.tile([B + 1, CO], BF16, name=f"arout{i}", tag=f"arout{i}") for i in range(2)]

            # ---- loads: merged big DMAs spread across engine queues ----
            for h in range(3):
                ts_, te_ = 3 * h, 3 * (h + 1)
                nc.sync.dma_start(
                    u1sb[:, ts_ * B:te_ * B].rearrange("p (t b) -> p t b", b=B),
                    u1d[ts_ * 128:te_ * 128, :].rearrange("(t p) b -> p t b", p=128))
                nc.gpsimd.dma_start(
                    w1sb[:, ts_ * CO:te_ * CO].rearrange("p (t f) -> p t f", f=CO),
                    w1d[ts_ * 128:te_ * 128, :].rearrange("(t p) f -> p t f", p=128))
            nc.scalar.dma_start(
                u2sb[:].rearrange("p (n f) -> p n f", f=IK),
                u2d[:, :].rearrange("(n p) f -> p n f", p=128))
            for g in range(2):
                nc.sync.dma_start(w2sb[g][:], w2d[g * 80:(g + 1) * 80, :])
                nc.gpsimd.dma_start(selsb[g][:], seld[g * 80:(g + 1) * 80, :])
            nc.scalar.dma_start(rsb[:], rmatd[:, :])
            nc.vector.memset(bsb[:], 0.0)
            nc.vector.memset(cexp[:], 1.0)
            nc.gpsimd.memset(onesb[:], 1.0)
            nc.gpsimd.memset(zrow[:], 0.0)
            nc.gpsimd.memset(qsb[:], 1.0 / float(I))   # iter-1 softmax 1/Z

            EXP = mybir.ActivationFunctionType.Exp
            SQUARE = mybir.ActivationFunctionType.Square
            SQRT = mybir.ActivationFunctionType.Sqrt

            def s_matmul(wt):
                stiles = [ps.tile([128, CO], F32, name="spsum", tag="pbig") for _ in range(NB)]
                for nb in range(NB):
                    for t in range(NT):
                        nc.tensor.matmul(
                            stiles[nb][:],
                            u1sb[:, t * B + nb * 128: t * B + (nb + 1) * 128],
                            wt[:, t * CO:(t + 1) * CO],
                            start=(t == 0), stop=(t == NT - 1))
                return stiles

            def z_matmul():
                z = ps2.tile([C, 1], F32, name="zpsum", tag="psmall")
                for t in range(NT):
                    nc.tensor.matmul(z[:], cexp[:, t * C:(t + 1) * C], onesb[:],
                                     start=(t == 0), stop=(t == NT - 1))
                return z

            def pack_and_allreduce(it, stiles, z):
                ain, aout = arin[it], arout[it]
                for nb in range(NB):
                    scol = slice(nb * CO, (nb + 1) * CO)
                    nc.vector.tensor_copy(spre[:, scol], stiles[nb][:])
                if z is not None:
                    nc.vector.tensor_copy(zpre[:], z[:])
                    nc.sync.dma_start(ain[B:B + 1, 0:C], zpre[:])
                    nc.sync.dma_start(ain[B:B + 1, C:CO], zrow[:, C:CO])
                else:
                    nc.sync.dma_start(ain[B:B + 1, :], zrow[:])
                nc.sync.dma_start(
                    ain[0:B, :].rearrange("(n p) f -> p n f", p=128),
                    spre[:].rearrange("p (n f) -> p n f", f=CO))
                # prefetch the Sqrt activation table while the collective
                # runs; actscr = spre[0,0]^2 ties it to pack time
                nc.vector.tensor_tensor(actscr[:], spre[0:1, 0:1],
                                        spre[0:1, 0:1], op=MULT)
                nc.scalar.activation(actscr[:], actscr[:], SQRT)
                nc.gpsimd.collective_compute(
                    "AllReduce", ADD,
                    replica_groups=[list(range(NCORES))],
                    ins=[ain[:].opt()], outs=[aout[:].opt()])
                # keep the PE HAM clock warm through the collective wait
                dums = ps.tile([128, 512], F32, name="dumw", tag="pbig")
                for _ in range(NDUMMY):
                    nc.tensor.matmul(dums[:], u1sb[:, 0:128], u1sb[:, 0:512],
                                     start=True, stop=True)
                # unpack (zbc first via gpsimd: q starts while ssb streams)
                if z is not None:
                    nc.sync.dma_start(zbc[:],
                                      aout[B:B + 1, 0:C].broadcast_to([128, C]))
                nc.sync.dma_start(
                    ssb[:].rearrange("p (n f) -> p n f", f=CO),
                    aout[0:B, :].rearrange("(n p) f -> p n f", p=128))

            def squash(const_q=False):
                # q = NCORES / Z8_glob  (= 1/Z); wide single ops over both
                # batch chunks at once.  Iter 1: Z == I exactly, q preset.
                if not const_q:
                    nc.vector.reciprocal(qsb[:], zbc[:])
                    nc.vector.tensor_scalar_mul(qsb[:], qsb[:], float(NCORES))
                s4 = ssb[:].rearrange("p (n c o) -> p n c o", c=C, o=O)
                q4 = qsb[:].unsqueeze(1).unsqueeze(3).broadcast_to([128, NB, C, O])
                nc.vector.tensor_tensor(s4, s4, q4, op=MULT)      # s = s~/Z
                nc.vector.tensor_tensor(sq[:], ssb[:], ssb[:], op=MULT)
                nc.vector.tensor_reduce(
                    n2[:],
                    sq[:].rearrange("p (n c o) -> p n o c", c=C, o=O),
                    axis=mybir.AxisListType.X, op=ADD)
                # phi = n2 / ((1+n2)*(sqrt(n2)+1e-10))
                nc.scalar.activation(phi[:], n2[:], SQRT)
                nc.vector.tensor_scalar_add(phi[:], phi[:], 1e-10)
                nc.vector.scalar_tensor_tensor(
                    phi[:], n2[:], 1.0, phi[:], op0=ADD, op1=MULT)
                nc.vector.reciprocal(phi[:], phi[:])
                nc.vector.tensor_tensor(phi[:], phi[:], n2[:], op=MULT)
                p4 = phi[:].rearrange("p (n o) -> p n o", o=O) \
                           .unsqueeze(2).broadcast_to([128, NB, C, O])
                o4 = outc[:].rearrange("p (n c o) -> p n c o", c=C, o=O)
                nc.vector.tensor_tensor(o4, s4, p4, op=MULT)
                # prefetch the Exp table (used by agr tail) while PE runs T
                nc.scalar.activation(actscr[:], phi[0:1, 0:1], EXP)

            def agr_phase():
                # T[co,ik] = sum_b out[b,co] u2[b,ik]; M = W2 .* T
                for g in range(2):
                    Tg = ps.tile([80, IK], F32, name="Tpsum", tag="pbig")
                    for c0, cn in ((0, 512), (512, 512), (1024, 128)):
                        for nb in range(NB):
                            nc.tensor.matmul(
                                Tg[:, c0:c0 + cn],
                                outc[:, nb * CO + g * 80: nb * CO + (g + 1) * 80],
                                u2sb[:, nb * IK + c0: nb * IK + c0 + cn],
                                start=(nb == 0), stop=(nb == NB - 1))
                    nc.vector.tensor_tensor(msb[g][:], w2sb[g][:], Tg[:], op=MULT)
                # all 9 tiles' agr_k into ONE psum tile (disjoint col ranges),
                # then single wide ops for copy / k-sum / b update / exp / cW
                agrk = ps2.tile([128, NT * C], F32, name="agrkp", tag="psmall")
                for t in range(NT):
                    tcol = slice(t * C, (t + 1) * C)
                    for g in range(2):
                        nc.tensor.matmul(agrk[:, tcol],
                                         msb[g][:, t * 128:(t + 1) * 128],
                                         selsb[g][:], start=(g == 0), stop=(g == 1))
                nc.vector.tensor_copy(agrk_sb[:], agrk[:])
                repl = ps2.tile([128, NT * C], F32, name="replp", tag="psmall")
                nc.tensor.matmul(repl[:], rsb[:], agrk_sb[:])
                nc.vector.tensor_tensor(bsb[:], bsb[:], repl[:], op=ADD)
                nc.scalar.activation(cexp[:], bsb[:], EXP)
                for h in range(3):
                    ts_, te_ = 3 * h, 3 * (h + 1)
                    cw4 = cwsb[:, ts_ * CO:te_ * CO].rearrange(
                        "p (t c o) -> p t c o", c=C, o=O)
                    w14 = w1sb[:, ts_ * CO:te_ * CO].rearrange(
                        "p (t c o) -> p t c o", c=C, o=O)
                    ce4 = cexp[:, ts_ * C:te_ * C].rearrange(
                        "p (t c) -> p t c", c=C) \
                        .unsqueeze(3).broadcast_to([128, 3, C, O])
                    nc.vector.tensor_tensor(cw4, w14, ce4, op=MULT)

            # ================= iteration 1 =================
            with nc.named_scope("it1_s"):
                stiles = s_matmul(w1sb)       # cexp == 1 -> cW == W1
            with nc.named_scope("it1_ar"):
                pack_and_allreduce(0, stiles, None)
            with nc.named_scope("it1_squash"):
                squash(const_q=True)
            with nc.named_scope("it1_agr"):
                agr_phase()
                z = z_matmul()
            # ================= iteration 2 =================
            with nc.named_scope("it2_s"):
                stiles = s_matmul(cwsb)
            with nc.named_scope("it2_ar"):
                pack_and_allreduce(1, stiles, z)
            with nc.named_scope("it2_squash"):
                squash()
            with nc.named_scope("it2_agr"):
                agr_phase()
                z = z_matmul()
            # ================= iteration 3 (s~ partial only) =================
            stiles = s_matmul(cwsb)
            for nb in range(NB):
                scol = slice(nb * CO, (nb + 1) * CO)
                nc.vector.tensor_copy(s3pre[:, scol], stiles[nb][:])
                nc.sync.dma_start(s3d[nb * 128:(nb + 1) * 128, :], s3pre[:, scol])
            nc.vector.tensor_copy(z3pre[:], z[:])
            nc.sync.dma_start(z3d[:, :], z3pre[:])

    nc.compile()
    return nc


def _get_nc():
    if "nc" not in _CACHE:
        _CACHE["nc"] = _build()
    return _CACHE["nc"]


def _host_inputs(x, weights):
    x = np.ascontiguousarray(x, dtype=np.float32)
    weights = np.ascontiguousarray(weights, dtype=np.float32)
    rmat = np.kron(np.eye(16, dtype=np.float32), np.ones((8, 8), np.float32))
    sel = np.zeros((CO, C), np.float32)
    for c in range(C):
        sel[c * O:(c + 1) * O, c] = 1.0 / B
    in_maps = []
    for m in range(NCORES):
        sl = slice(m * ILOC, (m + 1) * ILOC)
        xs = x[:, :, sl]                          # [B, K, ILOC]
        ws = weights[sl]                          # [ILOC, C, O, K]
        in_maps.append({
            "u1": np.ascontiguousarray(xs.transpose(2, 1, 0).reshape(IK, B)).astype(BF),
            "u2": np.ascontiguousarray(xs.transpose(0, 2, 1).reshape(B, IK)).astype(BF),
            "w1": np.ascontiguousarray(ws.transpose(0, 3, 1, 2).reshape(IK, CO)).astype(BF),
            "w2": np.ascontiguousarray(ws.transpose(1, 2, 0, 3).reshape(CO, IK)).astype(BF),
            "rmat": rmat.astype(BF),
            "sel": sel.astype(BF),
        })
    return in_maps


def kernel(x, weights):
    nc = _get_nc()
    in_maps = _host_inputs(x, weights)
    trace = bool(int(os.environ.get("KERNEL_TRACE", "0")))
    res = bass_utils.run_bass_kernel_spmd(
        nc, in_maps, core_ids=list(range(NCORES)), trace=trace)
    if trace and res.exec_time_ns is not None:
        print(f"HW exec time: {res.exec_time_ns} ns")
        _CACHE["exec_time_ns"] = res.exec_time_ns
        _CACHE["results"] = res
    s3 = np.zeros((B, CO), np.float64)
    z8 = np.zeros((C,), np.float64)
    for r in res.results:
        s3 += r["s3out"]
        z8 += r["z3out"][:, 0]
    s = (s3.reshape(B, C, O) / (z8 / NCORES)[None, :, None]).astype(np.float32)
    nsq = (s * s).sum(1, keepdims=True)
    out = s * (nsq / (1.0 + nsq)) / (np.sqrt(nsq) + 1e-10)
    return out[..., None].astype(np.float32)


# revision 19
# speedup vs baseline: 1.0748x; 1.0430x over previous
"""Trainium2 Bass kernel for the capsule-routing layer (nn_CapsConvLayer).

Math (reference):
  u_j_i[b,i,c,o] = sum_k W[i,c,o,k] * x[b,k,i]
  b_ic = 0
  3x:  c = softmax(b, axis=i)
       s[b,c,o]  = sum_i c[i,c] * u_j_i[b,i,c,o]
       out       = squash_over_c(s)
       agr[i,c]  = sum_{b,o} u_j_i[b,i,c,o] * out[b,c,o] / B
       b += agr
  return out[..., None]

Strategy: shard the input-capsule dim I across 8 cores (I_loc=144).  u_j_i is
never materialized; s is computed as one fused matmul with contraction over
(i,k) per core:  s~[b,co] = sum_ik u1[ik,b] * (exp(b)[i,c] * W1[ik,co]).
The softmax denominator Z and the partial s~ are all-reduced together
(the only cross-core traffic; 2 all-reduces total since the 3rd iteration's
agreement update is dead).  agr is computed locally per core via
T[co,ik] = sum_b out[b,co]*u2[b,ik] (PE), M = W2 .* T (DVE), then two small
PE contractions (class-selector and k-sum-replicate) put agr[i,c] directly
into the k-replicated [(i,k), c] layout used by the next iteration's weights.
The final iteration only needs partial s~3 + Z3, which the host sums and
squashes (tiny [256,10,16]).
"""

import os
import sys

sys.path.insert(0, "/opt/trn_rl_repo")

import ml_dtypes
import numpy as np

BF = ml_dtypes.bfloat16

import concourse.bacc as bacc
import concourse.mybir as mybir
import concourse.tile as tile
from concourse import bass_utils

# Problem constants (hardcoded per contract)
B, K, I, C, O = 256, 8, 1152, 10, 16
NCORES = 8
ILOC = I // NCORES          # 144
IK = ILOC * K               # 1152 contraction size per core
NT = IK // 128              # 9 partition tiles
CO = C * O                  # 160
NB = B // 128               # 2 batch chunks
F32 = mybir.dt.float32
BF16 = mybir.dt.bfloat16
ADD = mybir.AluOpType.add
MULT = mybir.AluOpType.mult

NDUMMY = 40

_CACHE = {}


def _build():
    nc = bacc.Bacc("TRN2", target_bir_lowering=False, debug=False,
                   enable_asserts=False, num_devices=NCORES)

    u1d = nc.dram_tensor("u1", [IK, B], BF16, kind="ExternalInput")
    u2d = nc.dram_tensor("u2", [B, IK], BF16, kind="ExternalInput")
    w1d = nc.dram_tensor("w1", [IK, CO], BF16, kind="ExternalInput")
    w2d = nc.dram_tensor("w2", [CO, IK], BF16, kind="ExternalInput")
    rmatd = nc.dram_tensor("rmat", [128, 128], BF16, kind="ExternalInput")
    seld = nc.dram_tensor("sel", [CO, C], BF16, kind="ExternalInput")
    s3d = nc.dram_tensor("s3out", [B, CO], F32, kind="ExternalOutput")
    z3d = nc.dram_tensor("z3out", [C, 1], F32, kind="ExternalOutput")

    with tile.TileContext(nc) as tc:
        with (
            tc.tile_pool(name="sb", bufs=1) as sb,
            tc.tile_pool(name="ps", bufs=2, space="PSUM") as ps,
            tc.tile_pool(name="ps2", bufs=2, space="PSUM") as ps2,
            tc.tile_pool(name="dram", bufs=1, space="DRAM") as dram,
        ):
            # persistent SBUF state
            u1sb = sb.tile([128, NT * B], BF16, tag="u1sb")     # [p, t*256+b]
            u2sb = sb.tile([128, NB * IK], BF16, tag="u2sb")    # [p, nb*1152+ik]
            w1sb = sb.tile([128, NT * CO], BF16, tag="w1sb")    # [p, t*160+co]
            w2sb = [sb.tile([80, IK], BF16, name=f"w2sb{g}", tag=f"w2sb{g}") for g in range(2)]
            cwsb = sb.tile([128, NT * CO], BF16, tag="cwsb")
            bsb = sb.tile([128, NT * C], F32, tag="bsb")       # logits, k-replicated
            cexp = sb.tile([128, NT * C], BF16, tag="cexp")
            onesb = sb.tile([128, 1], BF16, tag="onesb")
            rsb = sb.tile([128, 128], BF16, tag="rsb")
            selsb = [sb.tile([80, C], BF16, name=f"selsb{g}", tag=f"selsb{g}") for g in range(2)]
            ssb = sb.tile([128, NB * CO], BF16, tag="ssb")      # s~ global post-AR
            zbc = sb.tile([128, C], BF16, tag="zbc")
            qsb = sb.tile([128, C], F32, tag="qsb")
            outc = sb.tile([128, NB * CO], BF16, tag="outc")    # squashed caps
            agrk_sb = sb.tile([128, NT * C], BF16, tag="agrk_sb")
            msb = [sb.tile([80, IK], BF16, name=f"msb{g}", tag=f"msb{g}") for g in range(2)]
            zrow = sb.tile([1, CO], BF16, tag="zrow")
            spre = sb.tile([128, NB * CO], BF16, tag="spre")    # pre-AR s~ staging
            zpre = sb.tile([C, 1], BF16, tag="zpre")
            sq = sb.tile([128, NB * CO], BF16, tag="sq")
            n2 = sb.tile([128, NB * O], F32, tag="n2")
            phi = sb.tile([128, NB * O], F32, tag="phi")
            actscr = sb.tile([1, 1], F32, tag="actscr")
            s3pre = sb.tile([128, NB * CO], F32, tag="s3pre")
            z3pre = sb.tile([C, 1], F32, tag="z3pre")

            arin = [dram.tile([B + 1, CO], BF16, name=f"arin{i}", tag=f"arin{i}") for i in range(2)]
            arout = [dram.tile([B + 1, CO], BF16, name=f"arout{i}", tag=f"arout{i}") for i in range(2)]

            # ---- loads: merged big DMAs spread across engine queues ----
            for h in range(3):
                ts_, te_ = 3 * h, 3 * (h + 1)
                nc.sync.dma_start(
                    u1sb[:, ts_ * B:te_ * B].rearrange("p (t b) -> p t b", b=B),
                    u1d[ts_ * 128:te_ * 128, :].rearrange("(t p) b -> p t b", p=128))
                nc.gpsimd.dma_start(
                    w1sb[:, ts_ * CO:te_ * CO].rearrange("p (t f) -> p t f", f=CO),
                    w1d[ts_ * 128:te_ * 128, :].rearrange("(t p) f -> p t f", p=128))
            nc.scalar.dma_start(
                u2sb[:].rearrange("p (n f) -> p n f", f=IK),
                u2d[:, :].rearrange("(n p) f -> p n f", p=128))
            for g in range(2):
                nc.sync.dma_start(w2sb[g][:], w2d[g * 80:(g + 1) * 80, :])
                nc.gpsimd.dma_start(selsb[g][:], seld[g * 80:(g + 1) * 80, :])
            nc.scalar.dma_start(rsb[:], rmatd[:, :])
            nc.vector.memset(bsb[:], 0.0)
            nc.vector.memset(cexp[:], 1.0)
            nc.gpsimd.memset(onesb[:], 1.0)
            nc.gpsimd.memset(zrow[:], 0.0)
            nc.gpsimd.memset(qsb[:], 1.0 / float(I))   # iter-1 softmax 1/Z

            EXP = mybir.ActivationFunctionType.Exp
            SQUARE = mybir.ActivationFunctionType.Square
            SQRT = mybir.ActivationFunctionType.Sqrt

            def s_matmul(wt):
                stiles = [ps.tile([128, CO], F32, name="spsum", tag="pbig") for _ in range(NB)]
                for nb in range(NB):
                    for t in range(NT):
                        nc.tensor.matmul(
                            stiles[nb][:],
                            u1sb[:, t * B + nb * 128: t * B + (nb + 1) * 128],
                            wt[:, t * CO:(t + 1) * CO],
                            start=(t == 0), stop=(t == NT - 1))
                return stiles

            def z_matmul():
                z = ps2.tile([C, 1], F32, name="zpsum", tag="psmall")
                for t in range(NT):
                    nc.tensor.matmul(z[:], cexp[:, t * C:(t + 1) * C], onesb[:],
                                     start=(t == 0), stop=(t == NT - 1))
                return z

            def pack_and_allreduce(it, stiles, z):
                ain, aout = arin[it], arout[it]
                for nb in range(NB):
                    scol = slice(nb * CO, (nb + 1) * CO)
                    nc.vector.tensor_copy(spre[:, scol], stiles[nb][:])
                if z is not None:
                    nc.vector.tensor_copy(zpre[:], z[:])
                    nc.sync.dma_start(ain[B:B + 1, 0:C], zpre[:])
                    nc.sync.dma_start(ain[B:B + 1, C:CO], zrow[:, C:CO])
                else:
                    nc.sync.dma_start(ain[B:B + 1, :], zrow[:])
                nc.sync.dma_start(
                    ain[0:B, :].rearrange("(n p) f -> p n f", p=128),
                    spre[:].rearrange("p (n f) -> p n f", f=CO))
                # prefetch the Sqrt activation table while the collective
                # runs; actscr = spre[0,0]^2 ties it to pack time
                nc.vector.tensor_tensor(actscr[:], spre[0:1, 0:1],
                                        spre[0:1, 0:1], op=MULT)
                nc.scalar.activation(actscr[:], actscr[:], SQRT)
                nc.gpsimd.collective_compute(
                    "AllReduce", ADD,
                    replica_groups=[list(range(NCORES))],
                    ins=[ain[:].opt()], outs=[aout[:].opt()])

                # unpack (zbc first via gpsimd: q starts while ssb streams)
                if z is not None:
                    nc.sync.dma_start(zbc[:],
                                      aout[B:B + 1, 0:C].broadcast_to([128, C]))
                nc.sync.dma_start(
                    ssb[:].rearrange("p (n f) -> p n f", f=CO),
                    aout[0:B, :].rearrange("(n p) f -> p n f", p=128))

            def squash(const_q=False):
                # W1 is host-prescaled by 1/I, so iter-1 needs no q scale;
                # later iterations use q = I*NCORES/Z8_glob (= (1/I)/Ztrue).
                # Per batch-chunk so T matmuls can start on chunk 0 early.
                if not const_q:
                    nc.vector.reciprocal(qsb[:], zbc[:])
                    nc.vector.tensor_scalar_mul(qsb[:], qsb[:],
                                                float(NCORES) * float(I))
                for nb in range(NB):
                    scol = slice(nb * CO, (nb + 1) * CO)
                    ocol = slice(nb * O, (nb + 1) * O)
                    s4 = ssb[:, scol].rearrange("p (c o) -> p c o", o=O)
                    if not const_q:
                        q4 = qsb[:].unsqueeze(2).broadcast_to([128, C, O])
                        nc.vector.tensor_tensor(s4, s4, q4, op=MULT)
                    nc.vector.tensor_tensor(sq[:, scol], ssb[:, scol],
                                            ssb[:, scol], op=MULT)
                    nc.vector.tensor_reduce(
                        n2[:, ocol],
                        sq[:, scol].rearrange("p (c o) -> p o c", o=O),
                        axis=mybir.AxisListType.X, op=ADD)
                    # phi = n2 / ((1+n2)*(sqrt(n2)+1e-10))
                    nc.scalar.activation(phi[:, ocol], n2[:, ocol], SQRT)
                    nc.vector.tensor_scalar_add(phi[:, ocol], phi[:, ocol], 1e-10)
                    nc.vector.scalar_tensor_tensor(
                        phi[:, ocol], n2[:, ocol], 1.0, phi[:, ocol],
                        op0=ADD, op1=MULT)
                    nc.vector.reciprocal(phi[:, ocol], phi[:, ocol])
                    nc.vector.tensor_tensor(phi[:, ocol], phi[:, ocol],
                                            n2[:, ocol], op=MULT)
                    p4 = phi[:, ocol].unsqueeze(1).broadcast_to([128, C, O])
                    o4 = outc[:, scol].rearrange("p (c o) -> p c o", o=O)
                    nc.vector.tensor_tensor(o4, s4, p4, op=MULT)
                # prefetch the Exp table (used by agr tail) while PE runs T
                nc.scalar.activation(actscr[:], phi[0:1, 0:1], EXP)

            def agr_phase():
                # T[co,ik] = sum_b out[b,co] u2[b,ik]; M = W2 .* T
                Tgs = [ps.tile([80, IK], F32, name=f"Tpsum{g}", tag="pbig")
                       for g in range(2)]
                for nb in range(NB):
                    for g in range(2):
                        for c0, cn in ((0, 512), (512, 512), (1024, 128)):
                            nc.tensor.matmul(
                                Tgs[g][:, c0:c0 + cn],
                                outc[:, nb * CO + g * 80: nb * CO + (g + 1) * 80],
                                u2sb[:, nb * IK + c0: nb * IK + c0 + cn],
                                start=(nb == 0), stop=(nb == NB - 1),
                                skip_group_check=True)
                for g in range(2):
                    nc.vector.tensor_tensor(msb[g][:], w2sb[g][:], Tgs[g][:],
                                            op=MULT)
                # all 9 tiles' agr_k into ONE psum tile (disjoint col ranges),
                # then single wide ops for copy / k-sum / b update / exp / cW
                agrk = ps2.tile([128, NT * C], F32, name="agrkp", tag="psmall")
                for t in range(NT):
                    tcol = slice(t * C, (t + 1) * C)
                    for g in range(2):
                        nc.tensor.matmul(agrk[:, tcol],
                                         msb[g][:, t * 128:(t + 1) * 128],
                                         selsb[g][:], start=(g == 0), stop=(g == 1))
                nc.vector.tensor_copy(agrk_sb[:], agrk[:])
                repl = ps2.tile([128, NT * C], F32, name="replp", tag="psmall")
                nc.tensor.matmul(repl[:], rsb[:], agrk_sb[:])
                nc.vector.tensor_tensor(bsb[:], bsb[:], repl[:], op=ADD)
                nc.scalar.activation(cexp[:], bsb[:], EXP)
                for h in range(3):
                    ts_, te_ = 3 * h, 3 * (h + 1)
                    cw4 = cwsb[:, ts_ * CO:te_ * CO].rearrange(
                        "p (t c o) -> p t c o", c=C, o=O)
                    w14 = w1sb[:, ts_ * CO:te_ * CO].rearrange(
                        "p (t c o) -> p t c o", c=C, o=O)
                    ce4 = cexp[:, ts_ * C:te_ * C].rearrange(
                        "p (t c) -> p t c", c=C) \
                        .unsqueeze(3).broadcast_to([128, 3, C, O])
                    nc.vector.tensor_tensor(cw4, w14, ce4, op=MULT)

            # ================= iteration 1 =================
            with nc.named_scope("it1_s"):
                stiles = s_matmul(w1sb)       # cexp == 1 -> cW == W1
            with nc.named_scope("it1_ar"):
                pack_and_allreduce(0, stiles, None)
            with nc.named_scope("it1_squash"):
                squash(const_q=True)
            with nc.named_scope("it1_agr"):
                agr_phase()
                z = z_matmul()
            # ================= iteration 2 =================
            with nc.named_scope("it2_s"):
                stiles = s_matmul(cwsb)
            with nc.named_scope("it2_ar"):
                pack_and_allreduce(1, stiles, z)
            with nc.named_scope("it2_squash"):
                squash()
            with nc.named_scope("it2_agr"):
                agr_phase()
                z = z_matmul()
            # ================= iteration 3 (s~ partial only) =================
            stiles = s_matmul(cwsb)
            for nb in range(NB):
                scol = slice(nb * CO, (nb + 1) * CO)
                nc.vector.tensor_copy(s3pre[:, scol], stiles[nb][:])
                nc.sync.dma_start(s3d[nb * 128:(nb + 1) * 128, :], s3pre[:, scol])
            nc.vector.tensor_copy(z3pre[:], z[:])
            nc.sync.dma_start(z3d[:, :], z3pre[:])

    nc.compile()
    return nc


def _get_nc():
    if "nc" not in _CACHE:
        _CACHE["nc"] = _build()
    return _CACHE["nc"]


def _host_inputs(x, weights):
    x = np.ascontiguousarray(x, dtype=np.float32)
    weights = np.ascontiguousarray(weights, dtype=np.float32)
    rmat = np.kron(np.eye(16, dtype=np.float32), np.ones((8, 8), np.float32))
    sel = np.zeros((CO, C), np.float32)
    for c in range(C):
        sel[c * O:(c + 1) * O, c] = 1.0 / B
    in_maps = []
    for m in range(NCORES):
        sl = slice(m * ILOC, (m + 1) * ILOC)
        xs = x[:, :, sl]                          # [B, K, ILOC]
        ws = weights[sl]                          # [ILOC, C, O, K]
        in_maps.append({
            "u1": np.ascontiguousarray(xs.transpose(2, 1, 0).reshape(IK, B)).astype(BF),
            "u2": np.ascontiguousarray(xs.transpose(0, 2, 1).reshape(B, IK)).astype(BF),
            "w1": (np.ascontiguousarray(
                ws.transpose(0, 3, 1, 2).reshape(IK, CO)) / np.float32(I)).astype(BF),
            "w2": np.ascontiguousarray(ws.transpose(1, 2, 0, 3).reshape(CO, IK)).astype(BF),
            "rmat": rmat.astype(BF),
            "sel": sel.astype(BF),
        })
    return in_maps


def kernel(x, weights):
    nc = _get_nc()
    in_maps = _host_inputs(x, weights)
    trace = bool(int(os.environ.get("KERNEL_TRACE", "0")))
    res = bass_utils.run_bass_kernel_spmd(
        nc, in_maps, core_ids=list(range(NCORES)), trace=trace)
    if trace and res.exec_time_ns is not None:
        print(f"HW exec time: {res.exec_time_ns} ns")
        _CACHE["exec_time_ns"] = res.exec_time_ns
        _CACHE["results"] = res
    s3 = np.zeros((B, CO), np.float64)
    z8 = np.zeros((C,), np.float64)
    for r in res.results:
        s3 += r["s3out"]
        z8 += r["z3out"][:, 0]
    s = (s3.reshape(B, C, O) * (float(I) * NCORES / z8)[None, :, None]).astype(np.float32)
    nsq = (s * s).sum(1, keepdims=True)
    out = s * (nsq / (1.0 + nsq)) / (np.sqrt(nsq) + 1e-10)
    return out[..., None].astype(np.float32)
